# revision 22
# baseline (speedup 1.0000x reference)
"""Causal multi-head self-attention on 8 Trainium2 NeuronCores.

Problem (hardcoded): x [2, 2048, 1024] f32, Wq/Wk/Wv/Wo [1024, 1024] f32,
H=16 heads, Dh=64, causal softmax(QK^T/8)V then output projection.

Sharding (Megatron-style, per hint): 2-way data parallel over batch x
4-way tensor parallel over heads.  Core c handles batch c//4 and heads
4*(c%4) .. 4*(c%4)+3 (a 256-wide slice of the hidden dim).  Wq/Wk/Wv are
sliced column-wise, Wo row-wise; each core emits a partial [2048, 1024]
output which the host sums per batch (row-parallel unshard).

Device dataflow per core:
  - host supplies x^T (d on partitions) so QKV projections need no
    on-chip transpose
  - Q^T, K^T computed head-dim-on-partition; V seq-on-partition
  - scores computed transposed  S^T[k, q] with 2 heads packed in the PE
    array via row tiling (Dh=64 contraction)
  - causal mask added into PSUM via identity-matmul of host bf16 mask tiles
  - one exp() per [128, 1024] PSUM tile on ScalarE (scale=1/8 folded in;
    no max-subtraction: scores are ~N(0,1), exp never overflows)
  - A.V uses stationary [V | ones] so the softmax denominator appears as
    row 64 of the same matmul output
  - normalize: K=1 ones-matmul broadcasts denominators across partitions,
    fast DVE reciprocal, one tensor_tensor multiply
  - row-parallel Wo matmul

KCFG env selects matmul dtypes per stage (bf16 = 1 PE cycle/column,
float32r = 2): safe=all f32r, fast=bf16 except Wo, faster=all bf16.
"""

import os
import sys
from contextlib import ExitStack

import numpy as np

try:
    import concourse.bass as bass
except ImportError:  # pragma: no cover - path fallback for fresh dirs
    for p in ("/opt/trn_rl_repo", "/root/.axon_site/_ro/trn_rl_repo"):
        if os.path.isdir(p) and p not in sys.path:
            sys.path.insert(0, p)
    import concourse.bass as bass

import ml_dtypes
import concourse.bacc as bacc
import concourse.mybir as mybir
import concourse.tile as tile
from concourse.bass_utils import run_bass_kernel_spmd

F32 = mybir.dt.float32
F32R = mybir.dt.float32r
BF16 = mybir.dt.bfloat16

KCFG = os.environ.get("KCFG", "fast")
_DT = {
    "safe": dict(proj=F32R, qk=F32R, av=F32R, wo=F32R),
    "fast": dict(proj=BF16, qk=BF16, av=BF16, wo=F32R),
    "faster": dict(proj=BF16, qk=BF16, av=BF16, wo=BF16),
}[KCFG]

B, S, D = 2, 2048, 1024
H, DH = 16, 64
NCORES = 8
HPC = 4          # heads per core
JPC = HPC * DH   # 256 hidden dims per core
QB = 512         # query block (matmul moving free dim)
KB = 128         # key block (psum partition dim)
NQ = S // QB     # 4
NK = S // KB     # 16
MASK_VAL = -1e7

_CACHE = {}
LAST_RESULTS = None


def _np_dt(dt):
    return ml_dtypes.bfloat16 if dt == BF16 else np.float32


def _build_nc():
    proj_dt, qk_dt, av_dt, wo_dt = _DT["proj"], _DT["qk"], _DT["av"], _DT["wo"]
    nc = bacc.Bacc()
    xT = nc.dram_tensor("xT", [D, S], proj_dt, kind="ExternalInput")
    wqT = nc.dram_tensor("wqT", [D, JPC], proj_dt, kind="ExternalInput")
    wkT = nc.dram_tensor("wkT", [D, JPC], proj_dt, kind="ExternalInput")
    wvT = nc.dram_tensor("wvT", [D, JPC], proj_dt, kind="ExternalInput")
    woT = nc.dram_tensor("woT", [JPC, D], wo_dt, kind="ExternalInput")
    masks = nc.dram_tensor("masks", [4, KB, QB], BF16, kind="ExternalInput")
    ident = nc.dram_tensor("ident", [KB, KB], BF16, kind="ExternalInput")
    ones = nc.dram_tensor("ones", [KB, 64], av_dt, kind="ExternalInput")
    onesr = nc.dram_tensor("onesr", [1, 64], F32R, kind="ExternalInput")
    y = nc.dram_tensor("y", [S, D], F32, kind="ExternalOutput")

    with tile.TileContext(nc) as tc:
        with (
            tc.tile_pool(name="const", bufs=1) as constp,
            tc.tile_pool(name="act", bufs=1) as actp,
            tc.tile_pool(name="e", bufs=4) as ep,
            tc.tile_pool(name="ps", bufs=2, space="PSUM") as psp,
            tc.tile_pool(name="avp", bufs=4, space="PSUM") as avp,
        ):
            ident_sb = constp.tile([KB, KB], BF16)
            nc.sync.dma_start(out=ident_sb[:], in_=ident[:])
            masks_sb = constp.tile([KB, 4, QB], BF16)
            for d in range(4):
                nc.sync.dma_start(out=masks_sb[:, d, :], in_=masks[d, :, :])
            # ones row at partition 0 - stationary for K=1 broadcast matmuls
            ones_sb = constp.tile([1, 64], F32R)
            nc.sync.dma_start(out=ones_sb[:], in_=onesr[:])

            wo_sb = actp.tile([128, 2, D], wo_dt)
            for c in range(2):
                nc.sync.dma_start(
                    out=wo_sb[:, c, :], in_=woT[c * 128 : (c + 1) * 128, :]
                )

            # QT/KT: [128, S] pair tiles; rows 0:64 head 2*pi, 64:128 head 2*pi+1
            QT = [actp.tile([128, S], qk_dt, name=f"QT{i}") for i in range(2)]
            KT = [actp.tile([128, S], qk_dt, name=f"KT{i}") for i in range(2)]
            # V with ones column appended per (k-tile, head)
            V1 = actp.tile([128, NK, HPC, DH + 1], av_dt)
            nc.sync.dma_start(out=V1[:, :, :, DH : DH + 1], in_=ones[:, 0:NK * HPC])

            # ---------------- phase 1: QKV projections ----------------
            with tc.tile_pool(name="xw", bufs=1) as xwp:
                xT_sb = xwp.tile([128, 8, S], proj_dt)
                wq_sb = xwp.tile([128, 8, JPC], proj_dt)
                wk_sb = xwp.tile([128, 8, JPC], proj_dt)
                wv_sb = xwp.tile([128, 8, JPC], proj_dt)
                for dc in range(8):
                    dsl = slice(dc * 128, (dc + 1) * 128)
                    nc.sync.dma_start(out=xT_sb[:, dc, :], in_=xT[dsl, :])
                    nc.sync.dma_start(out=wq_sb[:, dc, :], in_=wqT[dsl, :])
                    nc.sync.dma_start(out=wk_sb[:, dc, :], in_=wkT[dsl, :])
                    nc.sync.dma_start(out=wv_sb[:, dc, :], in_=wvT[dsl, :])

                for w_sb, out_tiles in ((wq_sb, QT), (wk_sb, KT)):
                    for mj in range(2):
                        for qn in range(NQ):
                            ps = psp.tile([128, 1024], F32, tag="mm", name="ps_qk")
                            for dc in range(8):
                                nc.tensor.matmul(
                                    ps[:, :QB],
                                    lhsT=w_sb[:, dc, mj * 128 : (mj + 1) * 128],
                                    rhs=xT_sb[:, dc, qn * QB : (qn + 1) * QB],
                                    start=(dc == 0),
                                    stop=(dc == 7),
                                )
                            nc.vector.tensor_copy(
                                out_tiles[mj][:, qn * QB : (qn + 1) * QB],
                                ps[:, :QB],
                            )
                for st in range(NK):
                    ps = psp.tile([128, 1024], F32, tag="mm", name="ps_v")
                    for dc in range(8):
                        nc.tensor.matmul(
                            ps[:, :JPC],
                            lhsT=xT_sb[:, dc, st * 128 : (st + 1) * 128],
                            rhs=wv_sb[:, dc, :],
                            start=(dc == 0),
                            stop=(dc == 7),
                        )
                    nc.vector.tensor_copy(
                        V1[:, st, :, 0:DH],
                        ps[:, :JPC].rearrange("p (h d) -> p h d", h=HPC),
                    )

            # ---------------- phase 2: attention ----------------
            # late pool: opens after xw closed, reuses its SBUF space
            late_ctx = ExitStack()
            latep = late_ctx.enter_context(tc.tile_pool(name="late", bufs=1))
            OT = [latep.tile([128, S], wo_dt, name=f"OT{i}") for i in range(2)]
            # all softmax denominators live on partition 0, cols (head, q)
            sums_sb = latep.tile([1, HPC, S], F32R, name="sums_sb")

            for qn in range(NQ):
                av = [
                    avp.tile([DH + 1, QB], F32, tag="av", name=f"av{h}")
                    for h in range(HPC)
                ]
                nkt = 4 * qn + 4
                for kt in range(nkt):
                    straddle = kt >= 4 * qn
                    d = kt - 4 * qn
                    E = []
                    for pi in range(2):
                        ps = psp.tile([128, 1024], F32, tag="mm", name="ps_sc")
                        for hh in range(2):
                            nc.tensor.matmul(
                                ps[:, hh * QB : (hh + 1) * QB],
                                lhsT=KT[pi][
                                    hh * 64 : (hh + 1) * 64,
                                    kt * KB : (kt + 1) * KB,
                                ],
                                rhs=QT[pi][
                                    hh * 64 : (hh + 1) * 64,
                                    qn * QB : (qn + 1) * QB,
                                ],
                                start=True,
                                stop=not straddle,
                                tile_position=(hh * 64, 0),
                            )
                        if straddle:
                            for hh in range(2):
                                nc.tensor.matmul(
                                    ps[:, hh * QB : (hh + 1) * QB],
                                    lhsT=ident_sb,
                                    rhs=masks_sb[:, d, :],
                                    start=False,
                                    stop=True,
                                )
                        e = ep.tile([128, 1024], av_dt, tag="e", name="e")
                        nc.scalar.activation(
                            e[:], ps[:], mybir.ActivationFunctionType.Exp, scale=0.125
                        )
                        E.append(e)
                    for h in range(HPC):
                        pi, hh = h // 2, h % 2
                        nc.tensor.matmul(
                            av[h][:],
                            lhsT=V1[:, kt, h, :],
                            rhs=E[pi][:, hh * QB : (hh + 1) * QB],
                            start=(kt == 0),
                            stop=(kt == nkt - 1),
                        )
                for h in range(HPC):
                    pi, hh = h // 2, h % 2
                    nc.vector.tensor_copy(
                        OT[pi][hh * 64 : (hh + 1) * 64, qn * QB : (qn + 1) * QB],
                        av[h][0:DH, :],
                    )
                    nc.vector.tensor_copy(
                        sums_sb[0:1, h, qn * QB : (qn + 1) * QB],
                        av[h][DH : DH + 1, :],
                    )

            # ---------------- phase 3: softmax normalization ----------------
            for pi in range(2):
                for qn in range(NQ):
                    rb_ps = psp.tile([128, 1024], F32, tag="mm", name="rb_ps")
                    for hh in range(2):
                        nc.tensor.matmul(
                            rb_ps[0:64, hh * QB : (hh + 1) * QB],
                            lhsT=ones_sb[:],
                            rhs=sums_sb[0:1, 2 * pi + hh, qn * QB : (qn + 1) * QB],
                            start=True,
                            stop=True,
                        )
                    rb_sb = ep.tile([128, QB], F32, tag="rb", name="rb_sb")
                    for hh in range(2):
                        if os.environ.get("KRECIP", "slow") == "fast":
                            nc.vector.reciprocal_approx_fast(
                                out=rb_sb[hh * 64 : (hh + 1) * 64, :],
                                in_=rb_ps[0:64, hh * QB : (hh + 1) * QB],
                            )
                        else:
                            nc.vector.reciprocal(
                                rb_sb[hh * 64 : (hh + 1) * 64, :],
                                rb_ps[0:64, hh * QB : (hh + 1) * QB],
                            )
                    nc.vector.tensor_mul(
                        OT[pi][:, qn * QB : (qn + 1) * QB],
                        OT[pi][:, qn * QB : (qn + 1) * QB],
                        rb_sb[:],
                    )

            # ---------------- phase 4: output projection ----------------
            for st in range(NK):
                y_sb = latep.tile([128, D], F32, tag="y", bufs=3, name="y_sb")
                for nn in range(2):
                    ps = psp.tile([128, 1024], F32, tag="mm", name="ps_y")
                    for pi in range(2):
                        nc.tensor.matmul(
                            ps[:, :QB],
                            lhsT=OT[pi][:, st * 128 : (st + 1) * 128],
                            rhs=wo_sb[:, pi, nn * QB : (nn + 1) * QB],
                            start=(pi == 0),
                            stop=(pi == 1),
                        )
                    nc.scalar.copy(
                        y_sb[:, nn * QB : (nn + 1) * QB], ps[:, :QB]
                    )
                nc.sync.dma_start(
                    out=y[st * 128 : (st + 1) * 128, :], in_=y_sb[:]
                )
            late_ctx.close()
    return nc


def _get_nc():
    if "nc" not in _CACHE:
        nc = _build_nc()
        nc.finalize()  # Bacc lowering passes (wait split, reg alloc, ...)
        _CACHE["nc"] = nc
    return _CACHE["nc"]


def _host_consts():
    rk = np.arange(KB)[:, None]
    rq = np.arange(QB)[None, :]
    masks = np.empty((4, KB, QB), np.float32)
    for d in range(4):
        masks[d] = np.where(rq >= rk + 128 * d, 0.0, MASK_VAL)
    masks = masks.astype(ml_dtypes.bfloat16)
    identity = np.eye(KB, dtype=ml_dtypes.bfloat16)
    return masks, identity


def kernel(x, Wq, Wk, Wv, Wo):
    global LAST_RESULTS
    x = np.asarray(x, np.float32)
    Wq = np.asarray(Wq, np.float32)
    Wk = np.asarray(Wk, np.float32)
    Wv = np.asarray(Wv, np.float32)
    Wo = np.asarray(Wo, np.float32)

    pdt, wdt, adt = _np_dt(_DT["proj"]), _np_dt(_DT["wo"]), _np_dt(_DT["av"])
    masks, identity = _host_consts()
    ones_np = np.ones((KB, 64), adt)
    onesr_np = np.ones((1, 64), np.float32)
    xTs = [np.ascontiguousarray(x[b].T).astype(pdt) for b in range(B)]

    in_maps = []
    for c in range(NCORES):
        b, g = c // (NCORES // B), c % (NCORES // B)
        jsel = slice(g * JPC, (g + 1) * JPC)
        in_maps.append(
            {
                "xT": xTs[b],
                "wqT": np.ascontiguousarray(Wq[jsel].T).astype(pdt),
                "wkT": np.ascontiguousarray(Wk[jsel].T).astype(pdt),
                "wvT": np.ascontiguousarray(Wv[jsel].T).astype(pdt),
                "woT": np.ascontiguousarray(Wo[:, jsel].T).astype(wdt),
                "masks": masks,
                "ident": identity,
                "ones": ones_np,
                "onesr": onesr_np,
            }
        )

    res = run_bass_kernel_spmd(_get_nc(), in_maps, list(range(NCORES)))
    LAST_RESULTS = res
    ys = [res.results[c]["y"] for c in range(NCORES)]
    npc = NCORES // B
    out = np.stack(
        [sum(ys[b * npc + 1 : (b + 1) * npc], ys[b * npc]) for b in range(B)]
    )
    return out.astype(np.float32)
